# revision 4
# baseline (speedup 1.0000x reference)
"""Criss-cross (CCNet-style) sparse attention kernel for Trainium2.

Problem: B=8, C=512, H=W=96, CQ=64.
  q = Wq@x+bq, k = Wk@x+bk, v = Wv@x+bv  (1x1 convs)
  energy_H[h,w,g] = q[:,h,w].k[:,g,w] - 1e30*[h==g]   (column attention)
  energy_W[h,w,v'] = q[:,h,w].k[:,h,v']               (row attention)
  att = softmax(concat(energy_H, energy_W))           (per pixel, over H+W keys)
  out = gamma*(att_H @ v_col + att_W @ v_row) + x

Sharding: data-parallel over batch, one batch element per NeuronCore (8 cores).

Per-core plan (all phases under one TileContext):
  1. stream x -> q,k = Wqk@x (f32r matmuls, N=512 tiles), q/k stay in SBUF.
  2. energies per column/row (f32 matmuls, K=64, 96x96 outputs in PSUM),
     diag mask via +(-1e30*eye), stored f32 in SBUF; running per-pixel maxes.
  3. combined softmax stats (max over both directions via tiny PE transposes),
     exp via ScalarE with per-partition bias=-m and accum_out partial sums ->
     P_col/P_row in bf16; denominators -> R = 1/S (f32).
  4. re-stream x per image row h: v_row^T = x_row^T @ Wv^T (f32r) -> bf16;
     row attention applied: out_row_h = (P_row_h)^T.T @ v_row^T, scaled by R^T
     during PSUM->SBUF copy; v_row^T also scattered into vA (spatial-major v).
  5. column attention: out_col_w = P_colT_w.T @ vA[:,w,:], + row part, -> OFIN
     (channel-last DRAM scratch, bf16).
  6. DMA-transpose OFIN back to channel-major, out = x + gamma*attn + gamma*bv.
"""

import sys

if "/opt/trn_rl_repo" not in sys.path:
    sys.path.insert(0, "/opt/trn_rl_repo")

import numpy as np

B, C, HH, WW = 8, 512, 96, 96
CQ = 64
S = HH * WW  # 9216
NEG = np.float32(1e30)

_CACHE = {}


def _build():
    import concourse.bacc as bacc
    import concourse.tile as tile
    from concourse import mybir
    import ml_dtypes

    f32 = mybir.dt.float32
    f32r = mybir.dt.float32r
    bf16 = mybir.dt.bfloat16
    AF = mybir.ActivationFunctionType
    ALU = mybir.AluOpType
    AXX = mybir.AxisListType.X

    nc = bacc.Bacc("TRN2", target_bir_lowering=False)

    x_d = nc.dram_tensor("x", [C, S], f32r, kind="ExternalInput")
    wqkT_d = nc.dram_tensor("wqkT", [C, 2 * CQ], f32r, kind="ExternalInput")
    wvT_d = nc.dram_tensor("wvT", [C, C], f32r, kind="ExternalInput")
    bqk_d = nc.dram_tensor("bqk", [2 * CQ], f32, kind="ExternalInput")
    gbv_d = nc.dram_tensor("gbv", [C], f32, kind="ExternalInput")
    gam_d = nc.dram_tensor("gam", [1], f32, kind="ExternalInput")
    out_d = nc.dram_tensor("out", [C, S], f32, kind="ExternalOutput")

    ofin_d = nc.dram_tensor("ofin", [S, C], bf16)  # channel-last scratch
    orow_d = nc.dram_tensor("orow", [HH, WW, C], bf16)  # row-pass output scratch

    ident_bf_d = nc.inline_tensor(np.eye(96, dtype=ml_dtypes.bfloat16), name="idbf")
    ident_f_d = nc.inline_tensor(np.eye(96, dtype=np.float32), name="idf")
    mask_np = (-NEG * np.eye(96)).astype(np.float32)
    mask_d = nc.inline_tensor(mask_np, name="diagmask")

    with tile.TileContext(nc) as tc:
        with (
            tc.tile_pool(name="w", bufs=1) as pw,
            tc.tile_pool(name="pp", bufs=1) as ppp,
            tc.tile_pool(name="work", bufs=3) as pk,
            tc.tile_pool(name="ps", bufs=4, space="PSUM") as ps,
        ):
            # ---- constants / weights resident in SBUF ----
            wqk = pw.tile([128, 4, 2 * CQ], f32r)
            nc.sync.dma_start(wqk, wqkT_d[:, :].rearrange("(k p) m -> p k m", p=128))
            wv = pw.tile([128, 4, C], f32r)
            nc.sync.dma_start(wv, wvT_d[:, :].rearrange("(k p) m -> p k m", p=128))
            bqk = pw.tile([2 * CQ, 1], f32)
            nc.sync.dma_start(bqk, bqk_d[:].rearrange("(m o) -> m o", o=1))
            gbv = pw.tile([128, 4], f32)
            nc.sync.dma_start(gbv, gbv_d[:].rearrange("(k p) -> p k", p=128))
            gam = pw.tile([128, 1], f32)
            nc.gpsimd.dma_start(gam, gam_d[:].to_broadcast([128, 1]))
            idbf = pw.tile([96, 96], bf16)
            nc.sync.dma_start(idbf, ident_bf_d[:, :])
            idf = pw.tile([96, 96], f32)
            nc.sync.dma_start(idf, ident_f_d[:, :])
            mask = pw.tile([96, 96], f32)
            nc.sync.dma_start(mask, mask_d[:, :])

            # stats tiles (alive through phase 5)
            m_col = pw.tile([96, 96], f32)   # max over g of EC   [h, w]
            m_row = pw.tile([96, 96], f32)   # max over v' of ER  [w, h]
            neg_m = pw.tile([96, 96], f32)   # -(combined max)    [h, w]
            neg_mT = pw.tile([96, 96], f32)  # transposed         [w, h]
            s_col = pw.tile([96, 96], f32)   # sum exp col        [h, w]
            s_row = pw.tile([96, 96], f32)   # sum exp row        [w, h]
            rr = pw.tile([96, 96], f32)      # 1/denominator      [h, w]
            rrT = pw.tile([96, 96], f32)     # transposed         [w, h]

            # P tensors (bf16) alive phases 3-5
            p_col = ppp.tile([96, 96, 96], bf16)  # [h, w, g]
            p_row = ppp.tile([96, 96, 96], bf16)  # [w, h, v']

            with tc.tile_pool(name="qk", bufs=1) as pqk:
                q_sb = pqk.tile([CQ, S], f32)
                k_sb = pqk.tile([CQ, S], f32)

                # ---- phase 1: q, k projections ----
                NT = 512
                for st in range(S // NT):
                    xt = pk.tile([128, 4, NT], f32r, tag="xt1")
                    nc.sync.dma_start(
                        xt,
                        x_d[:, st * NT : (st + 1) * NT].rearrange(
                            "(k p) s -> p k s", p=128
                        ),
                    )
                    qk_ps = ps.tile([2 * CQ, NT], f32, tag="ops")
                    for ki in range(4):
                        nc.tensor.matmul(
                            qk_ps,
                            lhsT=wqk[:, ki, :],
                            rhs=xt[:, ki, :],
                            start=(ki == 0),
                            stop=(ki == 3),
                        )
                    nc.scalar.activation(
                        out=q_sb[:, st * NT : (st + 1) * NT],
                        in_=qk_ps[0:CQ, :],
                        func=AF.Identity,
                        bias=bqk[0:CQ, 0:1],
                        scale=1.0,
                    )
                    nc.scalar.activation(
                        out=k_sb[:, st * NT : (st + 1) * NT],
                        in_=qk_ps[CQ : 2 * CQ, :],
                        func=AF.Identity,
                        bias=bqk[CQ : 2 * CQ, 0:1],
                        scale=1.0,
                    )

                q3 = q_sb[:, :].rearrange("p (h w) -> p h w", w=96)
                k3 = k_sb[:, :].rearrange("p (h w) -> p h w", w=96)

                # ---- phase 2: energies (PSUM-resident) + per-pixel maxes ----
                for w in range(96):
                    e_ps = ps.tile([96, 96], f32, tag="eps")
                    nc.tensor.matmul(
                        e_ps, lhsT=q3[:, :, w], rhs=k3[:, :, w], start=True, stop=True
                    )
                    etmp = pk.tile([96, 96], f32, tag="etmp")
                    nc.vector.tensor_tensor(etmp, e_ps, mask, ALU.add)
                    nc.vector.reduce_max(m_col[:, w : w + 1], etmp, axis=AXX)
                for h in range(96):
                    e_ps = ps.tile([96, 96], f32, tag="eps")
                    nc.tensor.matmul(
                        e_ps, lhsT=q3[:, h, :], rhs=k3[:, h, :], start=True, stop=True
                    )
                    nc.vector.reduce_max(m_row[:, h : h + 1], e_ps, axis=AXX)

                t_ps = ps.tile([96, 96], f32, tag="eps")
                nc.tensor.transpose(t_ps, m_row, idf)  # -> [h, w]
                nc.vector.tensor_tensor(neg_m, m_col, t_ps, ALU.max)
                nc.vector.tensor_scalar_mul(neg_m, neg_m, -1.0)
                t_ps2 = ps.tile([96, 96], f32, tag="eps")
                nc.tensor.transpose(t_ps2, neg_m, idf)  # -> [w, h]
                nc.vector.tensor_copy(neg_mT, t_ps2)

                # ---- phase 3: exp (energies recomputed) ----
                for w in range(96):
                    e_ps = ps.tile([96, 96], f32, tag="eps")
                    nc.tensor.matmul(
                        e_ps, lhsT=q3[:, :, w], rhs=k3[:, :, w], start=True, stop=True
                    )
                    etmp = pk.tile([96, 96], f32, tag="etmp")
                    nc.vector.tensor_tensor(etmp, e_ps, mask, ALU.add)
                    nc.scalar.activation(
                        out=p_col[:, w, :],
                        in_=etmp,
                        func=AF.Exp,
                        bias=neg_m[:, w : w + 1],
                        scale=1.0,
                        accum_out=s_col[:, w : w + 1],
                    )
                for h in range(96):
                    e_ps = ps.tile([96, 96], f32, tag="eps")
                    nc.tensor.matmul(
                        e_ps, lhsT=q3[:, h, :], rhs=k3[:, h, :], start=True, stop=True
                    )
                    nc.scalar.activation(
                        out=p_row[:, h, :],
                        in_=e_ps,
                        func=AF.Exp,
                        bias=neg_mT[:, h : h + 1],
                        scale=1.0,
                        accum_out=s_row[:, h : h + 1],
                    )

            # denominators
            t_ps3 = ps.tile([96, 96], f32, tag="eps")
            nc.tensor.transpose(t_ps3, s_row, idf)  # -> [h, w]
            nc.vector.tensor_tensor(rr, s_col, t_ps3, ALU.add)
            nc.vector.reciprocal(rr, rr)
            t_ps4 = ps.tile([96, 96], f32, tag="eps")
            nc.tensor.transpose(t_ps4, rr, idf)  # -> [w, h]
            nc.vector.tensor_copy(rrT, t_ps4)

            ofin3 = ofin_d[:, :].rearrange("(h w) c -> h w c", w=96)

            orow3 = orow_d[:, :, :]
            with tc.tile_pool(name="vv", bufs=1) as pv:
                vA = pv.tile([96, 96, C], bf16)    # [g(h-dim), w, c] spatial-major v

                # ---- phase 4: v build + row attention ----
                x3 = x_d[:, :].rearrange("(k p) (h w) -> p k h w", p=128, w=96)
                for h in range(96):
                    xt = pk.tile([128, 4, 96], f32r, tag="xt4")
                    nc.sync.dma_start(xt, x3[:, :, h, :])
                    v_ps = ps.tile([96, C], f32, tag="ops")
                    for ki in range(4):
                        nc.tensor.matmul(
                            v_ps,
                            lhsT=xt[:, ki, :],
                            rhs=wv[:, ki, :],
                            start=(ki == 0),
                            stop=(ki == 3),
                        )
                    stg = pk.tile([96, C], bf16, tag="vstg")
                    nc.vector.tensor_copy(stg, v_ps)
                    nc.sync.dma_start(vA[h : h + 1, :, :], stg[:, :])
                    # transpose P_row[:, h, :] -> [v', w]
                    pt_ps = ps.tile([96, 96], bf16, tag="eps")
                    nc.tensor.transpose(pt_ps, p_row[:, h, :], idbf)
                    prT = pk.tile([96, 96], bf16, tag="prT")
                    nc.vector.tensor_copy(prT, pt_ps)
                    o_ps = ps.tile([96, C], f32, tag="ops")
                    nc.tensor.matmul(o_ps, lhsT=prT, rhs=stg, start=True, stop=True)
                    org = pk.tile([96, C], bf16, tag="org")
                    nc.scalar.activation(
                        out=org, in_=o_ps, func=AF.Copy, scale=rrT[:, h : h + 1]
                    )
                    nc.sync.dma_start(orow3[h, :, :], org[:, :])

                # ---- phase 5: column attention + combine ----
                for w in range(96):
                    pt_ps = ps.tile([96, 96], bf16, tag="eps")
                    nc.tensor.transpose(pt_ps, p_col[:, w, :], idbf)
                    pcT = pk.tile([96, 96], bf16, tag="prT")
                    nc.vector.tensor_copy(pcT, pt_ps)
                    o_ps = ps.tile([96, C], f32, tag="ops")
                    nc.tensor.matmul(
                        o_ps, lhsT=pcT, rhs=vA[:, w, :], start=True, stop=True
                    )
                    t1 = pk.tile([96, C], f32, tag="t1")
                    nc.scalar.activation(
                        out=t1, in_=o_ps, func=AF.Copy, scale=rr[:, w : w + 1]
                    )
                    rowt = pk.tile([96, C], bf16, tag="rowt")
                    nc.sync.dma_start(rowt, orow3[:, w, :])
                    of = pk.tile([96, C], bf16, tag="of")
                    nc.vector.tensor_tensor(of, t1, rowt, ALU.add)
                    nc.sync.dma_start(ofin3[:, w, :], of)

            # ---- phase 6: transpose back to channel-major, final add ----
            with tc.tile_pool(name="p6", bufs=2) as p6:
                NQ = 2304
                for ci in range(4):
                    for qt in range(S // NQ):
                        attn = p6.tile([128, NQ], bf16, tag="attn")
                        nc.sync.dma_start(
                            attn,
                            ofin_d[qt * NQ : (qt + 1) * NQ, ci * 128 : (ci + 1) * 128],
                            transpose=True,
                        )
                        xt = p6.tile([128, NQ], f32, tag="xt6")
                        nc.sync.dma_start(
                            xt,
                            x_d[ci * 128 : (ci + 1) * 128, qt * NQ : (qt + 1) * NQ].bitcast(f32),
                        )
                        t2 = p6.tile([128, NQ], f32, tag="t2")
                        nc.scalar.activation(
                            out=t2,
                            in_=attn,
                            func=AF.Identity,
                            bias=gbv[:, ci : ci + 1],
                            scale=gam[:, 0:1],
                        )
                        oo = p6.tile([128, NQ], f32, tag="oo")
                        nc.vector.tensor_add(oo, t2, xt)
                        nc.sync.dma_start(
                            out_d[ci * 128 : (ci + 1) * 128, qt * NQ : (qt + 1) * NQ],
                            oo,
                        )

    nc.compile()
    return nc


def _get_nc():
    if "nc" not in _CACHE:
        _CACHE["nc"] = _build()
    return _CACHE["nc"]


def kernel(x, Wq, bq, Wk, bk, Wv, bv, gamma):
    from concourse.bass_utils import run_bass_kernel_spmd

    x = np.asarray(x, np.float32)
    Wq = np.asarray(Wq, np.float32)
    Wk = np.asarray(Wk, np.float32)
    Wv = np.asarray(Wv, np.float32)
    bq = np.asarray(bq, np.float32)
    bk = np.asarray(bk, np.float32)
    bv = np.asarray(bv, np.float32)
    gamma = np.asarray(gamma, np.float32)

    nc = _get_nc()

    wqkT = np.ascontiguousarray(np.concatenate([Wq, Wk], axis=0).T)  # [C, 128]
    wvT = np.ascontiguousarray(Wv.T)  # [C, C]
    bqk = np.ascontiguousarray(np.concatenate([bq, bk]))  # [128]
    gbv = np.ascontiguousarray(gamma[0] * bv)  # [C]

    in_maps = []
    for b in range(B):
        in_maps.append(
            {
                "x": np.ascontiguousarray(x[b].reshape(C, S)),
                "wqkT": wqkT,
                "wvT": wvT,
                "bqk": bqk,
                "gbv": gbv,
                "gam": gamma,
            }
        )

    res = run_bass_kernel_spmd(nc, in_maps, core_ids=list(range(B)))
    out = np.stack([res.results[b]["out"].reshape(C, HH, WW) for b in range(B)])
    return out.astype(np.float32)
